# revision 26
# baseline (speedup 1.0000x reference)
"""Conv1d [16,512,4096] (x) * [512,512,5] (weight) + [512] (bias) -> [16,512,4096].

Strategy: data-parallel over batch across 8 NeuronCores (2 batches/core),
weight/bias replicated. Per core the conv is computed as 5 shifted matmuls
accumulated in PSUM:

  y[o, t] = bias[o] + sum_{k=0..4} sum_{c} wprep[k, c, o] * xpad[c, t + k]

with wprep[k, c, o] = weight[o, c, 4-k] (tap-flipped, transposed on host) and
xpad = x zero-padded by 2 along t. On the PE each out tile [128 o, 512 t]
accumulates 20 matmuls (4 c-chunks x 5 taps), lhsT = wprep chunk [128 c, 128 o]
stationary, rhs = shifted x slice [128 c, 512 t] moving, dtype float32r
(1 cycle/row at free-dim >= 256 vs 4 cycles/row for plain fp32).
"""

import numpy as np

B, C, O, T, K = 16, 512, 512, 4096, 5
PAD = 2
N_CORES = 8
BPC = B // N_CORES  # batches per core
CCH = C // 128      # c chunks
OCH = O // 128      # o chunks
TT = 512            # t tile (free dim; PSUM-bank/fp32-moving max)
NTT = T // TT       # t tiles per batch
NKC = K * CCH       # accumulating matmuls per out tile

_cached = {}

# Matmul operand dtype: "f32r" (TF32-like, rel err ~1.5e-4) or "bf16"
# (rel err ~2.4e-3, faster weight loads + half the input DMA bytes).
MM_DTYPE = "f32r"


def _build_nc():
    import concourse.bacc as bacc
    import concourse.bass as bass
    import concourse.mybir as mybir
    import concourse.tile as tile

    f32 = mybir.dt.float32
    f32r = mybir.dt.float32r if MM_DTYPE == "f32r" else mybir.dt.bfloat16

    nc = bacc.Bacc(None, target_bir_lowering=False, debug=False)

    SEG = 2 * TT + 2 * PAD  # x segment width: two t-tiles + halo
    XCOLS = (NTT // 2) * 2 * TT + 2 * PAD + 4  # padded x width (4104)

    x_dram = nc.dram_tensor("x", [BPC, C, XCOLS], f32r, kind="ExternalInput")
    # host layout: [k*CCH+cc, 128c, o]
    w_dram = nc.dram_tensor("w", [NKC, 128, O], f32r, kind="ExternalInput")
    b_dram = nc.dram_tensor("b", [128, OCH], f32, kind="ExternalInput")
    y_dram = nc.dram_tensor("y", [BPC, O, T], f32, kind="ExternalOutput")

    with tile.TileContext(nc) as tc:
        with (
            tc.tile_pool(name="wp", bufs=1) as wp,
            tc.tile_pool(name="bp", bufs=1) as bp,
            tc.tile_pool(name="xp", bufs=16 + 8) as xp,
            tc.tile_pool(name="pp", bufs=8, space=bass.MemorySpace.PSUM) as pp,
            tc.tile_pool(name="op", bufs=8) as op,
        ):
            # Two HWDGE queues (ACT + SP) fill in parallel against the HBM
            # roofline. The first out-tile consumes w chunks 0..19 in order,
            # so w is split: chunks 0-9 on ACT (start immediately), chunks
            # 10-19 on SP right after the first x segments they overlap
            # with. Output stores ride ACT (w is done before they start).
            w_all = wp.tile([128, NKC * O], f32r)
            bias_sb = bp.tile([128, OCH], f32)

            def load_w(i, eng):
                eng.dma_start(w_all[:, i * O:(i + 1) * O], w_dram[i])

            for i in range(NKC // 2):
                load_w(i, nc.scalar)

            seg = {}

            def load_x(b, j2, cc):
                xt = xp.tile([128, SEG], f32r, tag="xs")
                nc.sync.dma_start(
                    xt[:],
                    x_dram[b, cc * 128:(cc + 1) * 128,
                           j2 * 2 * TT:j2 * 2 * TT + SEG],
                )
                seg[(b, cc, j2)] = xt

            for cc in range(CCH):
                load_x(0, 0, cc)
            for i in range(NKC // 2, NKC):
                load_w(i, nc.sync)
            nc.scalar.dma_start(bias_sb[:], b_dram[:])

            for b in range(BPC):
                for j2 in range(NTT // 2):
                    for cc in range(CCH):
                        if (b, j2) != (0, 0):
                            load_x(b, j2, cc)

                for oc in range(OCH):
                    for j in range(NTT):
                        ps = pp.tile([128, TT], f32)
                        # accumulate in w-chunk DMA-arrival order so the very
                        # first out-tile's matmuls pipeline with the w loads
                        for ch in range(NKC):
                            k, cc = divmod(ch, CCH)
                            lhsT = w_all[:, ch * O + oc * 128:
                                         ch * O + oc * 128 + 128]
                            off = (j % 2) * TT + k
                            rhs = seg[(b, cc, j // 2)][:, off:off + TT]
                            nc.tensor.matmul(
                                ps[:], lhsT, rhs,
                                start=(ch == 0), stop=(ch == NKC - 1),
                            )
                        ot = op.tile([128, TT], f32)
                        nc.vector.tensor_scalar_add(
                            ot[:], ps[:], bias_sb[:, oc:oc + 1]
                        )
                        nc.scalar.dma_start(
                            y_dram[b, oc * 128:(oc + 1) * 128, j * TT:(j + 1) * TT],
                            ot[:],
                        )

    nc.finalize()
    return nc


def _get_nc():
    if "nc" not in _cached:
        _cached["nc"] = _build_nc()
    return _cached["nc"]


def run(x, weight, bias, trace=False):
    from concourse.bass_utils import run_bass_kernel_spmd

    nc = _get_nc()

    x = np.asarray(x, dtype=np.float32)
    weight = np.asarray(weight, dtype=np.float32)
    bias = np.asarray(bias, dtype=np.float32)

    # Zero halo: PAD cols left, PAD+4 right (rounds segment grid to 4104
    # cols), so the kernel needs no memsets.
    x = np.pad(x, ((0, 0), (0, 0), (PAD, PAD + 4)))
    if MM_DTYPE == "bf16":
        import ml_dtypes

        x = x.astype(ml_dtypes.bfloat16)

    # wprep[k, c, o] = weight[o, c, K-1-k]; chunked over c to [K*CCH, 128, O]
    wprep = np.ascontiguousarray(
        np.flip(weight, -1).transpose(2, 1, 0).reshape(NKC, 128, O)
    )
    if MM_DTYPE == "bf16":
        import ml_dtypes

        wprep = wprep.astype(ml_dtypes.bfloat16)
    bprep = np.ascontiguousarray(bias.reshape(OCH, 128).T)  # [128, OCH]

    in_maps = [
        {"x": x[i * BPC:(i + 1) * BPC], "w": wprep, "b": bprep}
        for i in range(N_CORES)
    ]
    res = run_bass_kernel_spmd(nc, in_maps, list(range(N_CORES)), trace=trace)
    y = np.concatenate([r["y"] for r in res.results], axis=0)
    return y, res


def kernel(x, weight, bias):
    y, _ = run(x, weight, bias)
    return y
